# revision 15
# baseline (speedup 1.0000x reference)
"""Trainium2 Bass kernel for ConvReshapeBefore (im2col patch extraction).

Full problem: x (32, 64, 64, 64) f32 NHWC, kernel 3x3 stride 1 valid ->
out (62*62*32, 3, 3, 64) f32 where out[(r*62+c)*32 + b] = x[b, r:r+3, c:c+3, :].

Sharding: data-parallel over batch, 4 batches per core across 8 cores.

Per-core pipeline:
  1. load x shard -> SBUF xt[p = h + 64*(b%2), free = (b//2)*4096 + w*64 + k]
     (even batches on the sync HWDGE ring -> partitions 0-63, odd batches
     on the scalar ring -> partitions 64-127; rings drain concurrently)
  2. PE matmuls (exact 0/1 routing): for c-chunk u, kernel-row i, batch b:
     psum[hf + r, (w', k)] = sum_h Id[h, r+i] * xt[h, (c0+w')*64+k]
     where hf = 64*(u%2): chunks alternate psum/stage partition halves.
  3. DVE+ACT copies expand the j-overlap (unsplit, 62 lanes):
     stage[hf+r, c*2304 + b*576 + i*192 + (jk)] = psum[hf+r, (c+j)*64+k]
     with the (j,k) dims merged into one 192-elem run whose source
     c-stride (64) overlaps -- the overlap IS the j-expansion.
  4. Stores: per chunk TWO 2D DMAs (rows 0-31 as 32 partitions, rows
     32-61 as 30) from the chunk's half.  Even chunks issue on the
     gpsimd SWDGE queue, odd chunks on the sync HWDGE queue: the two
     partition halves map to disjoint SDMA engine groups (~180 GB/s
     each), so the queues stream concurrently at ~350 GB/s aggregate.
     (One 62-partition dma runs at only ~56 GB/s; 64@32 is fast but
     requires padded DRAM rows.)
"""

import numpy as np

import concourse.bass as bass
import concourse.mybir as mybir
from concourse.ap import AP
from concourse.bass_utils import run_bass_kernel_spmd

# Full-problem constants (hardcoded per harness contract)
B, H, W, C = 32, 64, 64, 64
K = 3
R = H - K + 1  # 62
NCORES = 8
BS = B // NCORES  # 4

WC = W * C                    # 4096
ROW = 2 * WC                  # 8192 f32 per partition of xt
RUN = BS * K * K * C          # 2304 f32 per (r, c) output run
OUT_STRIDE_R = R * RUN        # 142848
CHUNKS = [(c0, min(6, R - c0)) for c0 in range(0, R, 6)]  # 11 chunks
NCH = len(CHUNKS)
NMM = NCH * K * BS            # 132 matmuls
BUF = 6 * RUN                 # f32 per stage buffer (3-deep ring per half)
PS = 3 * BUF                  # stage partition stride (f32)
PSROW = 4096                  # psum f32 per partition (8 banks x 512)


def _build_nc() -> bass.Bass:
    nc = bass.Bass(target_bir_lowering=False)
    x = nc.dram_tensor("x", [BS, H, W, C], mybir.dt.float32, kind="ExternalInput")
    out = nc.dram_tensor(
        "out", [R * R * BS, K, K, C], mybir.dt.float32, kind="ExternalOutput"
    )

    mms = [
        (u, i, b)
        for u in range(NCH)
        for i in range(K)
        for b in range(BS)
    ]

    # chunk u lives on partition half 64*(u%2), stage buffer (u//2)%2
    def half(u):
        return 64 * (u % 2)

    def sbuf(u):
        return ((u // 2) % 3) * BUF

    with (
        nc.sbuf_tensor("xt", [128, ROW], mybir.dt.float32) as xt,
        nc.sbuf_tensor("stage", [128, PS], mybir.dt.float32) as stage,
        nc.sbuf_tensor("iop", [128, 64], mybir.dt.float32) as iop,
        nc.sbuf_tensor("iof", [128, 64], mybir.dt.float32) as iof,
        nc.sbuf_tensor("ident", [128, 64], mybir.dt.float32) as ident,
        nc.psum_tensor("ps", [128, PSROW], mybir.dt.float32) as ps,
        nc.semaphore("l_e") as l_e,
        nc.semaphore("l_o") as l_o,
        nc.semaphore("isem") as isem,
        nc.semaphore("mm_sem") as mm_sem,
        nc.semaphore("cp0") as cp0,
        nc.semaphore("cp1") as cp1,
        nc.semaphore("st0") as st0,
        nc.semaphore("st1") as st1,
        nc.semaphore("st2") as st2,
        nc.semaphore("st3") as st3,
        nc.semaphore("st4") as st4,
        nc.semaphore("st5") as st5,
        nc.Block() as block,
    ):
        sts = (st0, st1, st2, st3, st4, st5)
        # total st incs per u%6 class: 32 per chunk
        st_tot = [32 * len([u for u in range(NCH) if u % 6 == q]) for q in range(6)]

        def copy_aps(n):
            # fused pair (n, n+1): same chunk, consecutive psum banks,
            # consecutive (i, b) stage offsets (affine outer dim)
            u, i, b = mms[n]
            u2, i2, b2 = mms[n + 1]
            assert u2 == u and (n % 8) + 1 == (n + 1) % 8
            c0, csz = CHUNKS[u]
            d1 = b * K * K * C + i * K * C
            d2 = b2 * K * K * C + i2 * K * C
            src = AP(
                ps,
                half(u) * PSROW + (n % 8) * 512,
                [[PSROW, R], [512, 2], [C, csz], [1, K * C]],
            )
            dst = AP(
                stage,
                half(u) * PS + sbuf(u) + d1,
                [[PS, R], [d2 - d1, 2], [RUN, csz], [1, K * C]],
            )
            return dst, src

        def store_aps(u, lo):
            # lo: rows 0-31 (32 partitions); else rows 32-61 (30 partitions)
            c0, csz = CHUNKS[u]
            p0, np_ = (0, 32) if lo else (32, 30)
            src = AP(
                stage,
                (half(u) + p0) * PS + sbuf(u),
                [[PS, np_], [1, csz * RUN]],
            )
            dst = AP(
                out,
                c0 * RUN + p0 * OUT_STRIDE_R,
                [[OUT_STRIDE_R, np_], [1, csz * RUN]],
            )
            return dst, src

        def load_aps(b):
            src = AP(x, b * H * WC, [[WC, H], [1, WC]])
            dst = AP(xt, (H * (b % 2)) * ROW + (b // 2) * WC, [[ROW, H], [1, WC]])
            return dst, src

        @block.sync
        def _(sync):
            for b in (0, 2):
                dst, src = load_aps(b)
                sync.dma_start(dst, src).then_inc(l_e, 16)
            for u in range(NCH):
                sync.wait_ge(cp0, 3 * (u + 1))
                sync.wait_ge(cp1, 3 * (u + 1))
                dst, src = store_aps(u, False)
                sync.dma_start(dst, src).then_inc(sts[u % 6], 16)
            for q in range(6):
                sync.wait_ge(sts[q], st_tot[q])

        @block.gpsimd
        def _(gp):
            gp.iota(
                AP(iop, 0, [[64, 128], [1, 64]]),
                [[0, 64]],
                channel_multiplier=1,
                allow_small_or_imprecise_dtypes=True,
            ).then_inc(isem, 1)
            gp.iota(
                AP(iof, 0, [[64, 64], [1, 64]]),
                [[1, 64]],
                channel_multiplier=0,
                allow_small_or_imprecise_dtypes=True,
            ).then_inc(isem, 1)
            gp.iota(
                AP(iof, 64 * 64, [[64, 64], [1, 64]]),
                [[1, 64]],
                base=64,
                channel_multiplier=0,
                allow_small_or_imprecise_dtypes=True,
            ).then_inc(isem, 1)
            for u in range(NCH):
                gp.wait_ge(cp0, 3 * (u + 1))
                gp.wait_ge(cp1, 3 * (u + 1))
                dst, src = store_aps(u, True)
                gp.dma_start(dst, src).then_inc(sts[u % 6], 16)
            for q in range(6):
                gp.wait_ge(sts[q], st_tot[q])

        @block.vector
        def _(vec):
            vec.wait_ge(isem, 3)
            vec.tensor_tensor(
                AP(ident, 0, [[64, 128], [1, 64]]),
                AP(iop, 0, [[64, 128], [1, 64]]),
                AP(iof, 0, [[64, 128], [1, 64]]),
                mybir.AluOpType.is_equal,
            ).then_inc(isem, 1)
            for n in range(0, NMM, 4):
                u = mms[n][0]
                vec.wait_ge(mm_sem, n + 2)
                if u >= 6:
                    vec.wait_ge(sts[u % 6], 32 * (u // 6))
                dst, src = copy_aps(n)
                vec.tensor_copy(dst, src).then_inc(cp0, 1)

        @block.scalar
        def _(sc):
            for b in (1, 3):
                dst, src = load_aps(b)
                sc.dma_start(dst, src).then_inc(l_o, 16)
            for n in range(2, NMM, 4):
                u = mms[n][0]
                sc.wait_ge(mm_sem, n + 2)
                if u >= 6:
                    sc.wait_ge(sts[u % 6], 32 * (u // 6))
                dst, src = copy_aps(n)
                sc.copy(dst, src).then_inc(cp1, 1)

        @block.tensor
        def _(te):
            te.wait_ge(isem, 4)
            for n in range(NMM):
                u, i, b = mms[n]
                c0, csz = CHUNKS[u]
                if n < BS:
                    te.wait_ge((l_e, l_o)[b % 2], 16 * (b // 2 + 1))
                if n >= 8:
                    # nearest earlier matmul sharing this psum bank AND half
                    j = n - 8
                    while j >= 0 and (mms[j][0] % 2) != (u % 2):
                        j -= 8
                    if j >= 0:
                        g = j // 2
                        te.wait_ge((cp0, cp1)[g % 2], g // 2 + 1)
                nfree = (csz + 2) * C
                out_ap = AP(
                    ps, half(u) * PSROW + (n % 8) * 512, [[PSROW, R], [1, nfree]]
                )
                lhsT = AP(ident, (b % 2) * H * 64 + i, [[64, 64], [1, R]])
                rhs = AP(
                    xt,
                    (b % 2) * H * ROW + (b // 2) * WC + c0 * C,
                    [[ROW, H], [C, csz + 2], [1, C]],
                )
                te.matmul(out_ap, lhsT, rhs).then_inc(mm_sem, 1)

    return nc


_NC = None


def _get_nc():
    global _NC
    if _NC is None:
        _NC = _build_nc()
    return _NC


def kernel(x: np.ndarray, **_run_kwargs) -> np.ndarray:
    assert x.shape == (B, H, W, C), x.shape
    nc = _get_nc()
    x = np.ascontiguousarray(x, dtype=np.float32)
    in_maps = [{"x": x[d * BS : (d + 1) * BS]} for d in range(NCORES)]
    res = run_bass_kernel_spmd(nc, in_maps, list(range(NCORES)), **_run_kwargs)
    outs = [res.results[d]["out"].reshape(R * R, BS, K, K, C) for d in range(NCORES)]
    full = np.concatenate(outs, axis=1).reshape(R * R * B, K, K, C)
    if _run_kwargs:
        return full, res
    return full
